# revision 20
# baseline (speedup 1.0000x reference)
"""Trainium2 Bass kernel for MoEST_Plus inference (spatial-transcriptomics
transformer: fourier pos-enc -> embed -> full self-attention -> top-1 MoE ->
gene decoder + func head), SPMD over 8 NeuronCores.

Sharding: data-parallel over the N=4096 spot dimension.  Each core owns a
512-token shard (queries / MoE / decoder / outputs) and re-computes the
embedding + K/V for the full token set locally (replication is cheaper than
collectives at this size).  Per-core inputs are *rolled* along the token axis
so that every core's own shard sits at columns 0:512 of its rolled inputs --
this keeps the compiled program identical across cores (pure SPMD, no
partition-id branching).  Attention is computed in "scores-transposed" layout
E[m,n] = exp(q_n . k_m / 8), m on partitions, which makes softmax
normalization a by-product of the PV matmul (ones-column trick) and needs no
transposes of the big attention matrix.

Activations are kept feature-major (z^T: [H, tok]) so every weight matmul
uses natural-layout weights; bf16 is used on the heavy matmul paths with fp32
accumulation/PSUM, fp32 for the router (argmax stability) and LN statistics.
"""

import os
import sys

for p in ("/opt/trn_rl_repo", "/opt/pypackages"):
    if p not in sys.path:
        sys.path.insert(0, p)

import math
from contextlib import ExitStack

import ml_dtypes
import numpy as np

import concourse.bass as bass
import concourse.tile as tile
from concourse import bacc, mybir
from concourse.masks import make_identity

F32 = mybir.dt.float32
BF16 = mybir.dt.bfloat16
AF = mybir.ActivationFunctionType
ALU = mybir.AluOpType
AX = mybir.AxisListType

N, DU, H, E, G = 4096, 1024, 256, 4, 2000
NH, HD = 4, 64
NCORES = 8
S = N // NCORES          # 512 tokens per core
C = N // 512             # token chunks (of 512) for full-N passes
PI = math.pi

bf16 = ml_dtypes.bfloat16


def _build_program():
    nc = bacc.Bacc(
        "TRN2",
        target_bir_lowering=False,
        debug=False,
        enable_asserts=False,
        num_devices=NCORES,
    )

    # ---------------- DRAM I/O ----------------
    def din(name, shape, dt):
        return nc.dram_tensor(name, list(shape), dt, kind="ExternalInput").ap()

    def dout(name, shape, dt):
        return nc.dram_tensor(name, list(shape), dt, kind="ExternalOutput").ap()

    visT = din("visT", (DU, N), BF16)          # rolled per core
    posT = din("posT", (3, N), F32)            # rolled per core
    gradT = din("gradT", (1, S), F32)          # own shard
    bfour = din("bfour", (3, 64), F32)
    wimg = din("wimg", (DU, H), BF16)
    wpos = din("wpos", (128, H), BF16)
    b_z = din("b_z", (128, 2), F32)            # b_img + b_pos, col per Mt
    wq = din("wq", (H, H), BF16)
    bq_ = din("bq", (128, 2), F32)
    wk = din("wk", (H, H), BF16)
    wv = din("wv", (H, H), BF16)
    wo = din("wo", (H, H), BF16)
    beff = din("beff", (128, 2), F32)          # bo + Wo^T bv
    g1 = din("g1", (128, 2), F32)
    b1 = din("b1", (128, 2), F32)
    wr = din("wr", (128, 8), F32)              # Wr[0:256] packed kt-major
    wr_g = din("wr_g", (1, 4), F32)
    br_ = din("br", (4, 1), F32)
    we1 = din("we1", (E, H, 4 * H), BF16)
    be1_ = din("be1", (128, 32), F32)          # col e*8+ft
    we2 = din("we2", (E, 4 * H, H), BF16)
    be2_ = din("be2", (128, 8), F32)           # col e*2+Mt
    wd1 = din("wd1", (H, H), BF16)
    bd1_ = din("bd1", (128, 2), F32)
    gd = din("gd", (128, 2), F32)
    bd_ = din("bd", (128, 2), F32)
    wd2e = din("wd2e", (H, G), BF16)           # even cols of Wd2
    bd2e = din("bd2e", (1, G), BF16)
    wf1 = din("wf1", (H, 64), BF16)
    bf1_ = din("bf1", (64, 1), F32)
    wf2 = din("wf2", (64, 1), BF16)
    nbf2 = din("nbf2", (1, 1), F32)            # -bf2

    mu_o = dout("mu", (S, G), F32)
    g_o = dout("gout", (S, 1), F32)
    pr_o = dout("probs", (S, E), F32)

    with tile.TileContext(nc) as tc, ExitStack() as ctx:
        P = ctx.enter_context(tc.tile_pool(name="persist", bufs=1))
        # PSUM: 8 banks total; pv (4 banks) is scoped to attention and a
        # 4-bank ps_x pool takes over for the post-attention stages.
        ps_sc = ctx.enter_context(tc.tile_pool(name="pssc", bufs=2, space="PSUM"))
        ps_mm = ctx.enter_context(tc.tile_pool(name="psmm", bufs=2, space="PSUM"))

        def sb(shape, dt, tag, pool=None):
            return (pool or P).tile(list(shape), dt, tag=tag, name=tag)

        # ---- constants ----
        ident_b = sb((128, 128), BF16, "ident_b")
        make_identity(nc, ident_b[:])
        ident_f = sb((128, 128), F32, "ident_f")
        make_identity(nc, ident_f[:])
        ones_row_b = sb((1, 128), BF16, "ones_row_b")
        nc.vector.memset(ones_row_b[:], 1.0)
        ones_row_f = sb((1, 128), F32, "ones_row_f")
        nc.vector.memset(ones_row_f[:], 1.0)
        ones_col_b = sb((128, 1), BF16, "ones_col_b")
        nc.vector.memset(ones_col_b[:], 1.0)
        cpi = sb((128, 1), F32, "cpi")
        nc.vector.memset(cpi[:], PI)
        ceps = sb((128, 1), F32, "ceps")
        nc.vector.memset(ceps[:], 1e-5)

        # ---- persistent weights/biases (small) ----
        def load(name, ap, shape, dt, pool=None):
            t = sb(shape, dt, name, pool)
            nc.sync.dma_start(t[:], ap)
            return t

        gradT_s = load("gradT", gradT[:, :], (1, S), F32)
        bfour_s = load("bfour", bfour[:, :], (3, 64), F32)
        wq_s = [load(f"wq{k}", wq[k * 128:(k + 1) * 128, :], (128, H), BF16)
                for k in range(2)]
        wk_s = [load(f"wk{k}", wk[k * 128:(k + 1) * 128, :], (128, H), BF16)
                for k in range(2)]
        wv_s = [load(f"wv{k}", wv[k * 128:(k + 1) * 128, :], (128, H), BF16)
                for k in range(2)]
        wo_s = [load(f"wo{k}", wo[k * 128:(k + 1) * 128, :], (128, H), BF16)
                for k in range(2)]
        b_z_s = load("b_z", b_z[:, :], (128, 2), F32)
        bq_s = load("bq", bq_[:, :], (128, 2), F32)
        beff_s = load("beff", beff[:, :], (128, 2), F32)
        g1_s = load("g1", g1[:, :], (128, 2), F32)
        b1_s = load("b1", b1[:, :], (128, 2), F32)
        wr_s = load("wr", wr[:, :], (128, 8), F32)
        wr_g_s = load("wr_g", wr_g[:, :], (1, 4), F32)
        br_s = load("br", br_[:, :], (4, 1), F32)

        # ---- persistent cross-stage activations ----
        z_f = [sb((128, 512), F32, f"z_f{m}") for m in range(2)]       # residual
        zln_f = [sb((128, 512), F32, f"zln_f{m}") for m in range(2)]
        zln_b = [sb((128, 512), BF16, f"zln_b{m}") for m in range(2)]
        zmoe = [sb((128, 512), F32, f"zmoe{m}") for m in range(2)]
        z2_b = [sb((128, 512), BF16, f"z2b{m}") for m in range(2)]
        wTe = [sb((1, 512), BF16, f"wTe{e}") for e in range(E)]
        wb_s = [sb((128, 512), BF16, f"wb{e}") for e in range(E)]
        d_b = [sb((128, 512), BF16, f"db{m}") for m in range(2)]

        # layer-norm over the feature (partition) dim via PE ones-matmuls,
        # rstd = exp(-0.5*ln(var+eps)) so only the exp/ln ACT table is used.
        def ln_feature_major(lnp, src_f, gamma, beta, out_f, out_b):
            srcb, sqb = [], []
            for m in range(2):
                t_b = lnp.tile([128, 512], BF16, tag="ln_b", name="ln_b", bufs=2)
                nc.vector.tensor_copy(t_b[:], src_f[m][:])
                srcb.append(t_b)
                t_q = lnp.tile([128, 512], BF16, tag="ln_q", name="ln_q", bufs=2)
                nc.scalar.square(t_q[:], src_f[m][:])
                sqb.append(t_q)
            ps_m = ps_mm.tile([1, 512], F32, tag="ps")
            for m in range(2):
                nc.tensor.matmul(ps_m[:], ones_col_b[:], srcb[m][:],
                                 start=(m == 0), stop=(m == 1))
            ps_q = ps_mm.tile([1, 512], F32, tag="ps")
            for m in range(2):
                nc.tensor.matmul(ps_q[:], ones_col_b[:], sqb[m][:],
                                 start=(m == 0), stop=(m == 1))
            mean = lnp.tile([1, 512], F32, tag="ln_mean", name="ln_mean", bufs=1)
            nc.vector.tensor_scalar(mean[:], ps_m[:], 1.0 / H, None, op0=ALU.mult)
            m2 = lnp.tile([1, 512], F32, tag="ln_m2", name="ln_m2", bufs=1)
            nc.vector.tensor_tensor(m2[:], mean[:], mean[:], op=ALU.mult)
            var = lnp.tile([1, 512], F32, tag="ln_var", name="ln_var", bufs=1)
            nc.vector.tensor_scalar(var[:], ps_q[:], 1.0 / H, None, op0=ALU.mult)
            nc.vector.tensor_tensor(var[:], var[:], m2[:], op=ALU.subtract)
            lnv = lnp.tile([1, 512], F32, tag="ln_lnv", name="ln_lnv", bufs=1)
            nc.scalar.activation(lnv[:], var[:], AF.Ln, bias=ceps[0:1, :])
            rstd = lnp.tile([1, 512], F32, tag="ln_rstd", name="ln_rstd", bufs=1)
            nc.scalar.activation(rstd[:], lnv[:], AF.Exp, scale=-0.5)
            cvec = lnp.tile([1, 512], F32, tag="ln_c", name="ln_c", bufs=1)
            nc.vector.tensor_tensor(cvec[:], mean[:], rstd[:], op=ALU.mult)
            ps_a = ps_mm.tile([128, 512], F32, tag="ps")
            nc.tensor.matmul(ps_a[:], ones_row_f[:], rstd[:], start=True, stop=True)
            a_b = lnp.tile([128, 512], F32, tag="ln_ab", name="ln_ab", bufs=1)
            nc.vector.tensor_copy(a_b[:], ps_a[:])
            ps_c = ps_mm.tile([128, 512], F32, tag="ps")
            nc.tensor.matmul(ps_c[:], ones_row_f[:], cvec[:], start=True, stop=True)
            c_b = lnp.tile([128, 512], F32, tag="ln_cb", name="ln_cb", bufs=1)
            nc.vector.tensor_copy(c_b[:], ps_c[:])
            for m in range(2):
                t1 = lnp.tile([128, 512], F32, tag="ln_t1", name="ln_t1", bufs=2)
                nc.vector.tensor_tensor(t1[:], src_f[m][:], a_b[:], op=ALU.mult)
                nc.vector.tensor_tensor(t1[:], t1[:], c_b[:], op=ALU.subtract)
                if out_f is not None:
                    nc.vector.tensor_scalar(out_f[m][:], t1[:], gamma[:, m:m + 1],
                                            beta[:, m:m + 1],
                                            op0=ALU.mult, op1=ALU.add)
                    if out_b is not None:
                        nc.gpsimd.tensor_copy(out_b[m][:], out_f[m][:])
                else:
                    nc.vector.tensor_scalar(out_b[m][:], t1[:], gamma[:, m:m + 1],
                                            beta[:, m:m + 1],
                                            op0=ALU.mult, op1=ALU.add)

        # ======== stage A/B window: embed + fourier + q/k/v ========
        with tc.tile_pool(name="embed", bufs=1) as emb, \
             tc.tile_pool(name="visstream", bufs=12) as vpool, \
             tc.tile_pool(name="attnbuf", bufs=1) as att, \
             tc.tile_pool(name="a0scr", bufs=2) as a0s:

            fourT = sb((128, N), BF16, "fourT", emb)
            for c in range(C):
                cs = slice(c * 512, (c + 1) * 512)
                post = a0s.tile([3, 512], F32, tag="pos", name="pos")
                nc.sync.dma_start(post[:], posT[:, cs])
                ps_xp = ps_mm.tile([64, 512], F32, tag="ps")
                nc.tensor.matmul(ps_xp[:], bfour_s[:], post[:],
                                 start=True, stop=True)
                # stack [t; t+0.25] into one [128,512] tile: sin rows give
                # sin(2*pi*t), cos rows give cos(2*pi*t) = sin(2*pi*(t+.25));
                # frac via f32->i32 round-to-nearest: arg in [-pi, pi]
                xps = a0s.tile([128, 512], F32, tag="xps", name="xps")
                nc.vector.tensor_copy(xps[0:64, :], ps_xp[:])
                nc.vector.tensor_scalar(xps[64:128, :], ps_xp[:], 0.25, None,
                                        op0=ALU.add)
                ri = a0s.tile([128, 512], mybir.dt.int32, tag="ri", name="ri")
                rf = a0s.tile([128, 512], F32, tag="rf", name="rf")
                nc.vector.tensor_copy(ri[:], xps[:])
                nc.gpsimd.tensor_copy(rf[:], ri[:])
                nc.vector.tensor_tensor(xps[:], xps[:], rf[:],
                                        op=ALU.subtract)
                nc.scalar.activation(fourT[:, cs], xps[:], AF.Sin,
                                     scale=2.0 * PI)

            wimg_s = [load(f"wimg{k}", wimg[k * 128:(k + 1) * 128, :],
                           (128, H), BF16, emb) for k in range(8)]
            wpos_s = load("wpos", wpos[:, :], (128, H), BF16, emb)
            z_b = [sb((128, N), BF16, f"z_b{m}", emb) for m in range(2)]
            for c in range(C):
                cs = slice(c * 512, (c + 1) * 512)
                vts = []
                for k in range(8):
                    vt = vpool.tile([128, 512], BF16, tag="vis", name="vis")
                    nc.sync.dma_start(vt[:], visT[k * 128:(k + 1) * 128, cs])
                    vts.append(vt)
                for m in range(2):
                    ms = slice(m * 128, (m + 1) * 128)
                    ps_z = ps_mm.tile([128, 512], F32, tag="ps")
                    for k in range(8):
                        nc.tensor.matmul(ps_z[:], wimg_s[k][:, ms], vts[k][:],
                                         start=(k == 0), stop=False)
                    nc.tensor.matmul(ps_z[:], wpos_s[:, ms], fourT[:, cs],
                                     start=False, stop=True)
                    nc.vector.tensor_scalar(z_b[m][:, cs], ps_z[:],
                                            b_z_s[:, m:m + 1], None, op0=ALU.add)
                    if c == 0:
                        nc.vector.tensor_scalar(z_f[m][:], ps_z[:],
                                                b_z_s[:, m:m + 1], None,
                                                op0=ALU.add)

            # ---- q (own), k (full), v (full, token-major + ones col) ----
            qT = [sb((128, 512), BF16, f"qT{m}", att) for m in range(2)]
            kT = [sb((128, N), BF16, f"kT{m}", att) for m in range(2)]
            for m in range(2):
                ms = slice(m * 128, (m + 1) * 128)
                ps_q = ps_mm.tile([128, 512], F32, tag="ps")
                for k in range(2):
                    nc.tensor.matmul(ps_q[:], wq_s[k][:, ms], z_b[k][:, 0:512],
                                     start=(k == 0), stop=(k == 1))
                nc.vector.tensor_scalar(qT[m][:], ps_q[:], bq_s[:, m:m + 1],
                                        None, op0=ALU.add)
                for c in range(C):
                    cs = slice(c * 512, (c + 1) * 512)
                    ps_k = ps_mm.tile([128, 512], F32, tag="ps")
                    for k in range(2):
                        nc.tensor.matmul(ps_k[:], wk_s[k][:, ms], z_b[k][:, cs],
                                         start=(k == 0), stop=(k == 1))
                    nc.vector.tensor_copy(kT[m][:, cs], ps_k[:])

            v65 = [sb((128, 260), BF16, f"v65_{mt}", att) for mt in range(32)]
            for mt in range(32):
                ts_ = slice(mt * 128, (mt + 1) * 128)
                ps_v = ps_mm.tile([128, 256], F32, tag="ps")
                for k in range(2):
                    nc.tensor.matmul(ps_v[:], z_b[k][:, ts_], wv_s[k][:],
                                     start=(k == 0), stop=(k == 1))
                vdst = v65[mt][:].rearrange("p (h x) -> p h x", h=4)[:, :, 0:64]
                vsrc = ps_v[:].rearrange("p (h d) -> p h d", h=4)
                nc.vector.tensor_copy(vdst, vsrc)
                vone = v65[mt][:].rearrange("p (h x) -> p h x", h=4)[:, :, 64:65]
                nc.vector.memset(vone, 1.0)

            # ======== attention ========
            with tc.tile_pool(name="epool", bufs=4) as epool, \
                 tc.tile_pool(name="aopool", bufs=1) as aop, \
                 tc.tile_pool(name="pspv", bufs=4, space="PSUM") as ps_pv, \
                 tc.tile_pool(name="ascr", bufs=4) as ascr:
                ao_all = [sb((128, 256), BF16, f"ao{nt}", aop)
                          for nt in range(4)]
                for h in range(NH):
                    hs = slice(64 * (h % 2), 64 * (h % 2) + 64)
                    vs_ = slice(65 * h, 65 * h + 65)
                    kTh = kT[h // 2]
                    qTh = qT[h // 2]
                    pv = [ps_pv.tile([128, 65], F32, tag="pv", name="pv")
                          for _ in range(4)]
                    for mt in range(32):
                        ts_ = slice(mt * 128, (mt + 1) * 128)
                        ps_s = ps_sc.tile([128, 512], F32, tag="sc")
                        nc.tensor.matmul(ps_s[:], kTh[hs, ts_], qTh[hs, :],
                                         start=True, stop=True)
                        Et = epool.tile([128, 512], BF16, tag="E", name="E")
                        nc.scalar.activation(Et[:], ps_s[:], AF.Exp,
                                             scale=0.125)
                        for nt in range(4):
                            nc.tensor.matmul(
                                pv[nt][:], Et[:, nt * 128:(nt + 1) * 128],
                                v65[mt][:, vs_],
                                start=(mt == 0), stop=(mt == 31))
                    for nt in range(4):
                        rec = ascr.tile([128, 1], F32, tag="rec", name="rec")
                        nc.vector.reciprocal(rec[:], pv[nt][:, 64:65])
                        nc.scalar.activation(ao_all[nt][:, 64 * h:64 * h + 64],
                                             pv[nt][:, 0:64], AF.Copy,
                                             scale=rec[:])

                # transpose ao (token-major) -> aoT (feature-major)
                aoT = [sb((128, 512), BF16, f"aoT{f}", aop) for f in range(2)]
                for nt in range(4):
                    for f in range(2):
                        ps_t = ps_mm.tile([128, 128], BF16, tag="ps")
                        nc.tensor.transpose(
                            ps_t[:], ao_all[nt][:, f * 128:(f + 1) * 128],
                            ident_b[:])
                        nc.vector.tensor_copy(
                            aoT[f][:, nt * 128:(nt + 1) * 128], ps_t[:])

                # ---- output proj + residual ----
                zres_f = []
                for m in range(2):
                    ms = slice(m * 128, (m + 1) * 128)
                    ps_at = ps_mm.tile([128, 512], F32, tag="ps")
                    for k in range(2):
                        nc.tensor.matmul(ps_at[:], wo_s[k][:, ms], aoT[k][:],
                                         start=(k == 0), stop=(k == 1))
                    zr = sb((128, 512), F32, f"zres{m}", aop)
                    nc.vector.tensor_scalar(zr[:], ps_at[:], beff_s[:, m:m + 1],
                                            None, op0=ALU.add)
                    nc.vector.tensor_tensor(zr[:], zr[:], z_f[m][:], op=ALU.add)
                    zres_f.append(zr)

                with tc.tile_pool(name="ln1", bufs=1) as lnp:
                    ln_feature_major(lnp, zres_f, g1_s, b1_s, zln_f, zln_b)

        # ======== router (fp32) ========
        with tc.tile_pool(name="router", bufs=2) as rp_:
            ps_lg = ps_mm.tile([4, 512], F32, tag="ps")
            nc.tensor.matmul(ps_lg[:], wr_s[:, 0:4], zln_f[0][:],
                             start=True, stop=False)
            nc.tensor.matmul(ps_lg[:], wr_s[:, 4:8], zln_f[1][:],
                             start=False, stop=False)
            nc.tensor.matmul(ps_lg[:], wr_g_s[:], gradT_s[:],
                             start=False, stop=True)
            lg_sb = rp_.tile([4, 512], F32, tag="lg", name="lg", bufs=1)
            nc.vector.tensor_scalar(lg_sb[:], ps_lg[:], br_s[:, 0:1], None,
                                    op0=ALU.add)
            for t4 in range(4):
                ts_ = slice(t4 * 128, (t4 + 1) * 128)
                ps_tr = ps_mm.tile([128, 4], F32, tag="ps")
                nc.tensor.matmul(ps_tr[:], lg_sb[:, ts_], ident_f[0:4, 0:4],
                                 is_transpose=True)
                lgt = rp_.tile([128, 4], F32, tag="lgt", name="lgt")
                nc.vector.tensor_copy(lgt[:], ps_tr[:])
                nmx = rp_.tile([128, 1], F32, tag="nmx", name="nmx")
                nc.vector.reduce_max(nmx[:], lgt[:], axis=AX.X, negate=True)
                e4 = rp_.tile([128, 4], F32, tag="e4", name="e4")
                se = rp_.tile([128, 1], F32, tag="se", name="se")
                nc.scalar.activation(e4[:], lgt[:], AF.Exp, bias=nmx[:],
                                     accum_out=se[:])
                rp2 = rp_.tile([128, 1], F32, tag="rp2", name="rp2")
                nc.vector.reciprocal(rp2[:], se[:])
                prt = rp_.tile([128, 4], F32, tag="prt", name="prt")
                nc.scalar.activation(prt[:], e4[:], AF.Copy, scale=rp2[:])
                nc.sync.dma_start(pr_o[t4 * 128:(t4 + 1) * 128, :], prt[:])
                val = rp_.tile([128, 1], F32, tag="val", name="val")
                nc.vector.reduce_max(val[:], prt[:], axis=AX.X)
                msk = rp_.tile([128, 4], F32, tag="msk", name="msk")
                nc.vector.tensor_scalar(msk[:], prt[:], val[:], None,
                                        op0=ALU.is_ge)
                wgt = rp_.tile([128, 4], BF16, tag="wgt", name="wgt")
                nc.vector.tensor_scalar(wgt[:], msk[:], val[:], None,
                                        op0=ALU.mult)
                for e in range(E):
                    ps_wt = ps_mm.tile([1, 128], BF16, tag="ps")
                    nc.tensor.matmul(ps_wt[:], wgt[:, e:e + 1], ident_b[:],
                                     is_transpose=True)
                    nc.vector.tensor_copy(wTe[e][:, ts_], ps_wt[:])

        # broadcast per-expert weights along partitions
        for e in range(E):
            ps_wb = ps_mm.tile([128, 512], F32, tag="ps")
            nc.tensor.matmul(ps_wb[:], ones_row_b[:], wTe[e][:],
                             start=True, stop=True)
            nc.vector.tensor_copy(wb_s[e][:], ps_wb[:])

        # late-stage weights (loaded here so startup DMA prioritizes embed)
        be1_s = load("be1", be1_[:, :], (128, 32), F32)
        be2_s = load("be2", be2_[:, :], (128, 8), F32)
        wd1_s = [load(f"wd1{k}", wd1[k * 128:(k + 1) * 128, :], (128, H), BF16)
                 for k in range(2)]
        bd1_s = load("bd1", bd1_[:, :], (128, 2), F32)
        gd_s = load("gd", gd[:, :], (128, 2), F32)
        bd_s = load("bd", bd_[:, :], (128, 2), F32)
        bd2e_s = load("bd2e", bd2e[:, :], (1, G), BF16)
        wf1_s = [load(f"wf1{k}", wf1[k * 128:(k + 1) * 128, :], (128, 64), BF16)
                 for k in range(2)]
        bf1_s = load("bf1", bf1_[:, :], (64, 1), F32)
        wf2_s = load("wf2", wf2[:, :], (64, 1), BF16)
        nbf2_s = load("nbf2", nbf2[:, :], (1, 1), F32)

        # ======== MoE experts (dense, blended by w_e) ========
        with tc.tile_pool(name="wepool", bufs=4) as wep, \
             tc.tile_pool(name="hbpool", bufs=9) as hbp, \
             tc.tile_pool(name="psx", bufs=4, space="PSUM") as ps_x, \
             tc.tile_pool(name="moescr", bufs=2) as mos:
            for e in range(E):
                we1t = []
                for k in range(2):
                    w1 = wep.tile([128, 4 * H], BF16, tag="we1", name="we1t",
                                  bufs=4)
                    nc.sync.dma_start(w1[:], we1[e, k * 128:(k + 1) * 128, :])
                    we1t.append(w1)
                we2t = []
                for f in range(8):
                    w2 = wep.tile([128, H], BF16, tag="we2", name="we2t",
                                  bufs=10)
                    nc.sync.dma_start(w2[:], we2[e, f * 128:(f + 1) * 128, :])
                    we2t.append(w2)
                hb = []
                for f in range(8):
                    fs = slice(f * 128, (f + 1) * 128)
                    ps_h = ps_x.tile([128, 512], F32, tag="px", name="px")
                    for k in range(2):
                        nc.tensor.matmul(ps_h[:], we1t[k][:, fs], zln_b[k][:],
                                         start=(k == 0), stop=(k == 1))
                    ht = hbp.tile([128, 512], BF16, tag="hb", name="hb")
                    nc.scalar.activation(ht[:], ps_h[:], AF.Gelu,
                                         bias=be1_s[:, e * 8 + f:e * 8 + f + 1])
                    hb.append(ht)
                for m in range(2):
                    ms = slice(m * 128, (m + 1) * 128)
                    ps_eo = ps_x.tile([128, 512], F32, tag="px", name="px")
                    for f in range(8):
                        nc.tensor.matmul(ps_eo[:], we2t[f][:, ms], hb[f][:],
                                         start=(f == 0), stop=(f == 7))
                    tmp = mos.tile([128, 512], F32, tag="eo", name="eo")
                    nc.vector.tensor_scalar(
                        tmp[:], ps_eo[:],
                        be2_s[:, e * 2 + m:e * 2 + m + 1], None, op0=ALU.add)
                    if e == 0:
                        nc.vector.tensor_tensor(zmoe[m][:], tmp[:], wb_s[e][:],
                                                op=ALU.mult)
                    else:
                        nc.vector.tensor_tensor(tmp[:], tmp[:], wb_s[e][:],
                                                op=ALU.mult)
                        nc.vector.tensor_tensor(zmoe[m][:], zmoe[m][:], tmp[:],
                                                op=ALU.add)

        for m in range(2):
            nc.vector.tensor_tensor(z2_b[m][:], zln_f[m][:], zmoe[m][:],
                                    op=ALU.add)

        # ======== gene decoder ========
        with tc.tile_pool(name="dec", bufs=1) as dec:
            dpre_f = []
            for m in range(2):
                ms = slice(m * 128, (m + 1) * 128)
                ps_d = ps_mm.tile([128, 512], F32, tag="ps")
                for k in range(2):
                    nc.tensor.matmul(ps_d[:], wd1_s[k][:, ms], z2_b[k][:],
                                     start=(k == 0), stop=(k == 1))
                dp = sb((128, 512), F32, f"dpre{m}", dec)
                nc.vector.tensor_scalar(dp[:], ps_d[:], bd1_s[:, m:m + 1],
                                        None, op0=ALU.add)
                dpre_f.append(dp)
            dln_f = [sb((128, 512), F32, f"dln{m}", dec) for m in range(2)]
            with tc.tile_pool(name="ln2", bufs=1) as lnp2:
                ln_feature_major(lnp2, dpre_f, gd_s, bd_s, dln_f, None)
            for m in range(2):
                nc.scalar.activation(d_b[m][:], dln_f[m][:], AF.Gelu)

        with tc.tile_pool(name="preds", bufs=1) as pre, \
             tc.tile_pool(name="psp", bufs=4, space="PSUM") as ps_p_pool:
            # ---- func head first: g = sigmoid(gelu(z2@Wf1+bf1)@Wf2+bf2) ----
            ps_f1 = ps_mm.tile([64, 512], F32, tag="ps")
            for k in range(2):
                nc.tensor.matmul(ps_f1[:], wf1_s[k][:], z2_b[k][:],
                                 start=(k == 0), stop=(k == 1))
            f1b = pre.tile([64, 512], BF16, tag="f1b", name="f1b")
            nc.scalar.activation(f1b[:], ps_f1[:], AF.Gelu, bias=bf1_s[:, 0:1])
            ps_g = ps_mm.tile([1, 512], F32, tag="ps")
            nc.tensor.matmul(ps_g[:], wf2_s[:], f1b[:], start=True, stop=True)
            # sigmoid(x+bf2) = 1/(1+exp(-x-bf2)) using the exp table
            em = pre.tile([1, 512], F32, tag="em", name="em")
            nc.scalar.activation(em[:], ps_g[:], AF.Exp, bias=nbf2_s[:, 0:1],
                                 scale=-1.0)
            nc.vector.tensor_scalar(em[:], em[:], 1.0, None, op0=ALU.add)
            gv = pre.tile([1, 512], F32, tag="gv", name="gv")
            nc.vector.reciprocal(gv[:], em[:])
            nc.sync.dma_start(g_o.rearrange("a b -> b a"), gv[:])

            wd2e_s = [load(f"wd2e{k}", wd2e[k * 128:(k + 1) * 128, :],
                           (128, G), BF16, pre) for k in range(2)]
            # softplus(x) = ln(1+exp(x)); batch ALL exp passes then ALL ln
            # passes so the ACT table isn't reloaded per tile (exp and ln
            # live in different greedy-chosen table sets).
            spts = []
            exp_insts = []
            for t4 in range(4):
                ts_ = slice(t4 * 128, (t4 + 1) * 128)
                for ch in range(4):
                    chs = slice(ch * 500, (ch + 1) * 500)
                    ps_p = ps_p_pool.tile([128, 500], F32, tag="pp",
                                          name="pp")
                    nc.tensor.matmul(ps_p[:], d_b[0][:, ts_], wd2e_s[0][:, chs],
                                     start=True, stop=False)
                    nc.tensor.matmul(ps_p[:], d_b[1][:, ts_], wd2e_s[1][:, chs],
                                     start=False, stop=False)
                    nc.tensor.matmul(ps_p[:], ones_row_b[:], bd2e_s[:, chs],
                                     start=False, stop=True)
                    spt = pre.tile([128, 500], BF16, tag="spt", name="spt",
                                   bufs=16)
                    exp_insts.append(nc.scalar.activation(spt[:], ps_p[:],
                                                          AF.Exp))
                    spts.append((spt, ts_, chs))
            # force all ln passes after all exp passes (one ACT table swap
            # instead of one per tile -- the scheduler is table-blind)
            for spt, ts_, chs in spts:
                mu_t = pre.tile([128, 500], F32, tag="mu", name="mu",
                                bufs=3)
                ln_i = nc.scalar.activation(mu_t[:], spt[:], AF.Ln, bias=1.0)
                tile.add_dep_helper(ln_i.ins, exp_insts[-1].ins, sync=False,
                                    reason="act table phase order")
                nc.sync.dma_start(mu_o[ts_, chs], mu_t[:])

    nc.compile()
    return nc


_CACHE = {}


def _get_program():
    if "nc" not in _CACHE:
        _CACHE["nc"] = _build_program()
    return _CACHE["nc"]


def kernel(vis, pos, grad, B_four, W_img, b_img, W_pos, b_pos,
           Wq, bq, Wk, bk, Wv, bv, Wo, bo, ln1_g, ln1_b,
           Wr, br, We1, be1, We2, be2,
           Wd1, bd1, lnd_g, lnd_b, Wd2, bd2,
           Wf1, bf1, Wf2, bf2):
    from concourse import bass_utils

    nc = _get_program()

    f32 = np.float32
    vis = np.asarray(vis, f32)
    visT_full = np.ascontiguousarray(vis.T).astype(bf16)          # [DU, N]
    posT_full = np.ascontiguousarray(np.asarray(pos, f32).T)      # [3, N]
    grad = np.asarray(grad, f32)

    def b(x):
        return np.ascontiguousarray(np.asarray(x, f32)).astype(bf16)

    def f(x):
        return np.ascontiguousarray(np.asarray(x, f32))

    def packcols(v, width=128):
        # [K] -> [128, K/128] column-per-ktile
        v = np.asarray(v, f32)
        return np.ascontiguousarray(v.reshape(-1, width).T)

    Wo_f = np.asarray(Wo, f32)
    beff = np.asarray(bo, f32) + np.asarray(bv, f32) @ Wo_f
    b_zv = np.asarray(b_img, f32) + np.asarray(b_pos, f32)

    common = {
        "bfour": f(B_four),
        "wimg": b(W_img),
        "wpos": b(W_pos),
        "b_z": packcols(b_zv),
        "wq": b(Wq), "bq": packcols(bq),
        "wk": b(Wk),
        "wv": b(Wv),
        "wo": b(Wo),
        "beff": packcols(beff),
        "g1": packcols(ln1_g), "b1": packcols(ln1_b),
        # rows 0:128 of Wr -> cols 0:4, rows 128:256 -> cols 4:8
        "wr": np.ascontiguousarray(
            np.asarray(Wr, f32)[0:256].reshape(2, 128, 4)
            .transpose(1, 0, 2).reshape(128, 8)),
        "wr_g": f(np.asarray(Wr, f32)[256:257]),
        "br": f(np.asarray(br, f32).reshape(4, 1)),
        "we1": b(We1),
        "be1": np.ascontiguousarray(
            np.asarray(be1, f32).reshape(E, 8, 128).transpose(2, 0, 1)
            .reshape(128, 32)),
        "we2": b(We2),
        "be2": np.ascontiguousarray(
            np.asarray(be2, f32).reshape(E, 2, 128).transpose(2, 0, 1)
            .reshape(128, 8)),
        "wd1": b(Wd1), "bd1": packcols(bd1),
        "gd": packcols(lnd_g), "bd": packcols(lnd_b),
        "wd2e": b(np.asarray(Wd2, f32).reshape(H, G, 2)[:, :, 0]),
        "bd2e": b(np.asarray(bd2, f32).reshape(G, 2)[:, 0].reshape(1, G)),
        "wf1": b(Wf1),
        "bf1": f(np.asarray(bf1, f32).reshape(64, 1)),
        "wf2": b(Wf2),
        "nbf2": f(-np.asarray(bf2, f32).reshape(1, 1)),
    }

    in_maps = []
    for i in range(NCORES):
        s = i * S
        in_map = dict(common)
        in_map["visT"] = np.ascontiguousarray(
            np.concatenate([visT_full[:, s:], visT_full[:, :s]], axis=1))
        in_map["posT"] = np.ascontiguousarray(
            np.concatenate([posT_full[:, s:], posT_full[:, :s]], axis=1))
        in_map["gradT"] = np.ascontiguousarray(grad[s:s + S, 0].reshape(1, S))
        in_maps.append(in_map)

    res = bass_utils.run_bass_kernel_spmd(nc, in_maps, list(range(NCORES)))
    mu = np.concatenate([r["mu"] for r in res.results], axis=0)
    g = np.concatenate([r["gout"] for r in res.results], axis=0)
    probs = np.concatenate([r["probs"] for r in res.results], axis=0)
    return mu.astype(np.float32), g.astype(np.float32), probs.astype(np.float32)


if __name__ == "__main__":
    nc = _get_program()
    print("compiled OK")
